# revision 19
# baseline (speedup 1.0000x reference)
"""Trainium2 Bass kernel for the water-balance NN recurrence.

Per-step math (per basin b):
    h       = relu(W1[0]*inflow + W1[1]*storage + b1)      # (256,)
    outflow = W2 . h + b2
    storage' = storage + DT*(inflow - outflow)

Implementation (per core, 8192 basins, data-parallel over 8 cores):
  - Basins processed in rounds of CH=4 chunks x N=256 basins.  Chunk c of a
    round occupies the 32-partition strip starting at 32c, so the small-K
    (K=3) layer-1 matmuls and small-M (M=3) layer-2 matmuls pack into
    disjoint tile_position strips of the PE array and run concurrently.
  - X tile per round r (SBUF [128, N]), rows within strip c:
      +0 inflow_t   +1 storage_t   +2 const 1.0 (bias row)
  - Layer 1: matmul lhsT=[3,128] (W1 half + b1 row, replicated per strip),
    rhs = X[32c:32c+3, :] -> PSUM hp [128 hidden, 2N] (two hidden halves).
  - relu: ACT/DVE PSUM->SBUF hs.
  - Layer 2 + state update fused on the PE into PSUM tile sp, rows:
      +0 = b2 + W2.h                              = outflow_t
      +1 = storage + DT*inflow - DT*b2 - DT*W2.h  = storage_{t+1}
      +2 = 1.0                                    (regenerated ones row)
    via 3 accumulating matmuls per chunk: affine (K=3, M=3, fp32) and two
    W2-half matmuls (K=128, M=3, lhsT = [W2h, -DT*W2h, 0]).
  - One full-tile copy sp -> X_next moves storage'+ones into place for the
    next step (row +0 gets outflow, immediately overwritten by the inflow
    prefetch DMA).  Outputs leave via one [2,N] DMA per chunk from X_next
    rows (+0,+1) into an interleaved (outflow, storage') HBM tensor that the
    host de-interleaves.  All DMA/engine access patterns are
    partition-contiguous.
  - Time loop is a hardware For_i loop with an unrolled body.
"""

import os
from contextlib import ExitStack

import numpy as np

T = 512
B = 65536
N_CORES = 8
HID = 256
DT_WB = 0.1

B_CORE = B // N_CORES      # 8192
N = 256                    # basins per chunk (matmul free dim)
CH = 4                     # chunks per round (PE strips)
R = B_CORE // (N * CH)     # 8 rounds per step
P = 128

USE_F32R = os.environ.get("WB_F32R", "1") == "1"
UNROLL = int(os.environ.get("WB_UNROLL", "8"))


def _prep_weights(W1, b1, W2, b2):
    """Host-side packing of the tiny MLP into PE-friendly layouts."""
    W1 = np.asarray(W1, np.float32)
    b1 = np.asarray(b1, np.float32)
    W2 = np.asarray(W2, np.float32).reshape(HID)
    b2f = float(np.asarray(b2, np.float32).reshape(()))

    w1a = np.zeros((P, P), np.float32)
    w1b = np.zeros((P, P), np.float32)
    # layer-2 stationary operands are [K, 32] so the packed matmuls write a
    # full 32-row strip of sp (cols 3..31 zero) -> no uninitialized PSUM rows
    aff = np.zeros((P, 32), np.float32)
    for c in range(CH):
        s = 32 * c
        w1a[s + 0] = W1[0, :128]
        w1a[s + 1] = W1[1, :128]
        w1a[s + 2] = b1[:128]
        w1b[s + 0] = W1[0, 128:]
        w1b[s + 1] = W1[1, 128:]
        w1b[s + 2] = b1[128:]
        aff[s + 0, 1] = DT_WB                    # inflow row
        aff[s + 1, 1] = 1.0                      # storage row
        aff[s + 2, 0] = b2f                      # ones row -> outflow col
        aff[s + 2, 1] = -DT_WB * b2f
        aff[s + 2, 2] = 1.0
    w2a = np.zeros((P, 32), np.float32)
    w2b = np.zeros((P, 32), np.float32)
    w2a[:, 0] = W2[:128]
    w2a[:, 1] = -DT_WB * W2[:128]
    w2b[:, 0] = W2[128:]
    w2b[:, 1] = -DT_WB * W2[128:]
    return {"w1a": w1a, "w1b": w1b, "w2a": w2a, "w2b": w2b, "aff": aff}


def _prep_xinit(storage0, n_rounds=R):
    """[CH, 2, R*N] chunk-major (storage0, ones) pairs for the initial X."""
    storage0 = np.asarray(storage0, np.float32)
    rw = n_rounds * N
    xi = np.empty((CH, 2, rw), np.float32)
    xi[:, 0, :] = storage0.reshape(CH, rw)
    xi[:, 1, :] = 1.0
    return np.ascontiguousarray(xi.reshape(-1))


def _build(ctx, tc, ins, outs, *, n_steps=T, n_rounds=R, unroll=UNROLL,
           use_f32r=USE_F32R, dyn_loop=True):
    import concourse.bass as bass
    import concourse.mybir as mybir

    nc = tc.nc
    f32 = mybir.dt.float32
    f32r = mybir.dt.float32r

    def mm_cast(ap):
        return ap.bitcast(f32r) if use_f32r else ap

    inflows = ins["inflows"]     # [n_steps+1, n_rounds*CH*N]
    xinit = ins["xinit"]         # [n_rounds*CH*2*N]
    w1a_d, w1b_d = ins["w1a"], ins["w1b"]
    w2a_d, w2b_d = ins["w2a"], ins["w2b"]
    aff_d = ins["aff"]
    comb = outs["comb"]          # [n_steps, n_rounds*CH*2*N] (outflow,storage')

    const = ctx.enter_context(tc.tile_pool(name="const", bufs=1))
    state = ctx.enter_context(tc.tile_pool(name="state", bufs=1))
    hp_pool = ctx.enter_context(tc.tile_pool(name="hp", bufs=6, space="PSUM"))
    sp_pool = ctx.enter_context(tc.tile_pool(name="sp", bufs=2, space="PSUM"))
    hs_pool = ctx.enter_context(tc.tile_pool(name="hs", bufs=6))
    stgo_pool = ctx.enter_context(tc.tile_pool(name="stgo", bufs=2))
    stgi_pool = ctx.enter_context(tc.tile_pool(name="stgi", bufs=2))

    w1a_t = const.tile([P, P], f32, tag="w1a")
    w1b_t = const.tile([P, P], f32, tag="w1b")
    w2a_t = const.tile([P, 32], f32, tag="w2a")
    w2b_t = const.tile([P, 32], f32, tag="w2b")
    aff_t = const.tile([P, 32], f32, tag="aff")
    nc.sync.dma_start(w1a_t[:], w1a_d[:])
    nc.sync.dma_start(w1b_t[:], w1b_d[:])
    nc.sync.dma_start(w2a_t[:], w2a_d[:])
    nc.sync.dma_start(w2b_t[:], w2b_d[:])
    nc.sync.dma_start(aff_t[:], aff_d[:])

    # ping-pong X tiles: one [128, R*N] tile per parity; round r occupies
    # free-dim columns [r*N, (r+1)*N).  Basin (within core) = c*R*N + r*N + n.
    RW = n_rounds * N
    X = [state.tile([P, RW], f32, tag=f"x{p}", name=f"x{p}") for p in (0, 1)]
    xinit_v = xinit.rearrange("(c k w) -> c k w", k=2, w=RW)
    for c in range(CH):
        s = 32 * c
        nc.sync.dma_start(X[0][s + 1:s + 3, :], xinit_v[c])
        nc.sync.dma_start(X[0][s:s + 1, :],
                          inflows[0:1, c * RW:(c + 1) * RW])

    def step_body(t_sym, parity):
        Xc, Xn = X[parity], X[parity ^ 1]
        for r in range(n_rounds):
            col = slice(r * N, (r + 1) * N)
            # layer 1: per chunk, 2 hidden halves -> hp_c [128 hidden, 2N]
            # (4 chunks in disjoint row strips run concurrently on the PE)
            hps = []
            for c in range(CH):
                s = 32 * c
                hp = hp_pool.tile([P, 2 * N], f32, tag="hp", name=f"hp{c}")
                for half, w1t in ((0, w1a_t), (1, w1b_t)):
                    nc.tensor.matmul(
                        hp[:, half * N:(half + 1) * N],
                        mm_cast(w1t[s:s + 3, :]),
                        mm_cast(Xc[s:s + 3, col]),
                        start=True, stop=True,
                        tile_position=(s, 0),
                    )
                hps.append(hp)

            # relu PSUM->SBUF, split over ACT and DVE
            hss = []
            for c, hp in enumerate(hps):
                hs = hs_pool.tile([P, 2 * N], f32, tag="hs", name=f"hs{c}")
                if c < 2:
                    nc.scalar.activation(hs[:], hp[:],
                                         mybir.ActivationFunctionType.Relu)
                else:
                    nc.vector.tensor_scalar_max(hs[:], hp[:], 0.0)
                hss.append(hs)

            # layer 2 + state update -> sp strip rows
            #   {+0: outflow, +1: storage', +2: ones}
            sp = sp_pool.tile([P, N], f32, tag="sp")
            for c in range(CH):
                s = 32 * c
                hs = hss[c]
                nc.tensor.matmul(
                    sp[s:s + 32, :], aff_t[s:s + 3, :], Xc[s:s + 3, col],
                    start=True, stop=False, tile_position=(s, s),
                )
                nc.tensor.matmul(
                    sp[s:s + 32, :], mm_cast(w2a_t[:]), mm_cast(hs[:, 0:N]),
                    start=False, stop=False, tile_position=(0, s),
                )
                nc.tensor.matmul(
                    sp[s:s + 32, :], mm_cast(w2b_t[:]), mm_cast(hs[:, N:2 * N]),
                    start=False, stop=True, tile_position=(0, s),
                )

            # state hand-off: full-height copy puts storage'+ones into X_next
            # (row +0 gets outflow; the inflow prefetch overwrites it)
            nc.scalar.copy(Xn[:, col], sp[:])

        # whole-step I/O through staging tiles: exactly one dynamic
        # (bounds-checked) DMA per direction per step -- the per-chunk
        # scatter/gather is done with static SBUF->SBUF DMAs.
        stg_o = stgo_pool.tile([8, RW], f32, tag="stgo", name="stg_o")
        for c in range(CH):
            s = 32 * c
            nc.sync.dma_start(stg_o[2 * c:2 * c + 2, :], Xn[s:s + 2, :])
        nc.sync.dma_start(comb[bass.ds(t_sym, 1), :], stg_o[:])

        stg_i = stgi_pool.tile([CH, RW], f32, tag="stgi", name="stg_i")
        nc.gpsimd.dma_start(stg_i[:], inflows[bass.ds(t_sym + 1, 1), :])
        for c in range(CH):
            s = 32 * c
            nc.gpsimd.dma_start(Xn[s:s + 1, :], stg_i[c:c + 1, :])

    if dyn_loop:
        import concourse.mybir as mybir_
        assert n_steps % unroll == 0 and unroll % 2 == 0
        hint = ((mybir_.EngineType.PE,)
                if os.environ.get("WB_HINT", "0") == "1" else ())
        with tc.For_i(0, n_steps, unroll, hint_engines=hint) as iv:
            for u in range(unroll):
                step_body(iv + u, u & 1)
    else:
        for t in range(n_steps):
            step_body(t, t & 1)


def _run_hw(inflows, storage0, W1, b1, W2, b2):
    import concourse.bass as bass
    import concourse.bacc as bacc
    import concourse.tile as tile
    import concourse.mybir as mybir
    from concourse.bass_utils import run_bass_kernel_spmd

    f32 = mybir.dt.float32
    nc_b = bacc.Bacc("TRN2", num_devices=N_CORES)
    d = {}
    d["inflows"] = nc_b.dram_tensor("inflows", [T + 1, B_CORE], f32,
                                    kind="ExternalInput").ap()
    d["xinit"] = nc_b.dram_tensor("xinit", [R * CH * 2 * N], f32,
                                  kind="ExternalInput").ap()
    for nm, shp in (("w1a", [P, P]), ("w1b", [P, P]), ("w2a", [P, 32]),
                    ("w2b", [P, 32]), ("aff", [P, 32])):
        d[nm] = nc_b.dram_tensor(nm, shp, f32, kind="ExternalInput").ap()
    outs = {"comb": nc_b.dram_tensor("comb", [T, R * CH * 2 * N], f32,
                                     kind="ExternalOutput").ap()}

    with tile.TileContext(nc_b) as tc, ExitStack() as ctx:
        _build(ctx, tc, d, outs, dyn_loop=True)
    nc_b.compile()

    wts = _prep_weights(W1, b1, W2, b2)
    inflows2 = np.asarray(inflows, np.float32).reshape(T, B)
    storage0 = np.asarray(storage0, np.float32)
    in_maps = []
    for c in range(N_CORES):
        sl = slice(c * B_CORE, (c + 1) * B_CORE)
        infl = np.concatenate(
            [inflows2[:, sl], np.zeros((1, B_CORE), np.float32)], axis=0)
        m = {"inflows": np.ascontiguousarray(infl),
             "xinit": _prep_xinit(storage0[sl])}
        m.update(wts)
        in_maps.append(m)

    res = run_bass_kernel_spmd(nc_b, in_maps, core_ids=list(range(N_CORES)))

    storages = np.empty((T + 1, B, 1), np.float32)
    outflows = np.empty((T, B, 1), np.float32)
    for c in range(N_CORES):
        sl = slice(c * B_CORE, (c + 1) * B_CORE)
        cb = res.results[c]["comb"].reshape(T, CH, 2, R * N)
        outflows[:, sl, 0] = cb[:, :, 0, :].reshape(T, B_CORE)
        storages[1:, sl, 0] = cb[:, :, 1, :].reshape(T, B_CORE)
    storages[0, :, 0] = storage0
    return storages, outflows


def kernel(inflows, storage0, W1, b1, W2, b2):
    return _run_hw(inflows, storage0, W1, b1, W2, b2)
